# revision 36
# baseline (speedup 1.0000x reference)
"""LBP-5x3 + 59-bin histogram kernel for TRN2 (8 NeuronCores, data parallel).

Full inputs: x [128, 512, 512] fp32 in [0,1). Output: [128, 59] fp32.
Each core processes 16 images as 8 PAIRS: partitions 0..63 hold image A
(8 row-blocks of 64), partitions 64..127 image B, so every elementwise
pass covers two images and per-instruction overheads halve. Per pair:
  u8 = floor(x*255)  (RNE(x*255-0.5) on device; exact-int cases fixed on host)
  8 neighbor compares (zero-padded) -> weighted sum on PE -> LBP code 0..255
  59-bin uniform histogram: 44 bins via DVE is_equal+accum passes; the 14
  bins in the consecutive-code runs 0..4 / 126..129 / 251..255 via single
  ACT Sign+accum passes whose cumulative-count chain is solved on the host
  (one DVE is_lt pass anchors the interior run). Per-partition accumulators
  keep the two images separable. Counts are mod-256 (uint8 wrap, host).
"""
import sys

sys.path.insert(0, "/opt/trn_rl_repo")
sys.path.insert(0, "/opt/pypackages")

import numpy as np

import concourse.bacc as bacc
import concourse.tile as tile
from concourse import mybir
from concourse.bass_utils import run_bass_kernel_spmd
from concourse.masks import make_identity

# (bin index of run start, code, run length, anchor kind)
CUM_RUNS = [(0, 0, 5, "start"), (27, 126, 4, "anchor"), (53, 251, 5, "end")]

UNIS = np.array([0, 1, 2, 3, 4, 6, 7, 8, 12, 14, 15, 16, 24, 28, 30, 31, 32, 48, 56,
                 60, 62, 63, 64, 96, 112, 120, 124, 126, 127, 128, 129, 131, 135, 143,
                 159, 191, 192, 193, 195, 199, 207, 223, 224, 225, 227, 231, 239, 240,
                 241, 243, 247, 248, 249, 251, 252, 253, 254, 255], dtype=np.int32)

# (dy, dx, weight): neighbor at img[y+dy, x+dx] compared >= img[y, x]
NEIGHBORS = [(-3, 0, 1), (-3, 3, 2), (0, 5, 4), (3, 3, 8),
             (3, 0, 16), (3, -3, 32), (0, -5, 64), (-3, -3, 128)]

NIMG = 16          # images per core
NPAIR = 8
H = W = 512
NB = 8             # row blocks of 64 per image half
PB = 64            # rows per block (= partitions per image half)
BW = 528           # block width with halo (8 left, 8 right)
OFF = 8            # image col offset inside a block
FW = NB * BW       # full free width of haloed tiles (4224)
CW = NB * W        # full free width of compact tiles (4096)
NBIN = len(UNIS)   # 58
NPIX = float(H * W)

F32 = mybir.dt.float32
BF16 = mybir.dt.bfloat16
I16 = mybir.dt.int16

_CACHE = {}


def _img3(t, start, width=W):
    """3D AP over a haloed [128, FW] tile: blocks x width cols from `start`."""
    return t[:].rearrange("p (b c) -> p b c", b=NB)[:, :, start:start + width]


def _blockv(t, p0, n, b0, nb):
    """AP over partitions [p0, p0+n) x full blocks [b0, b0+nb) of a haloed
    tile (each block is BW wide)."""
    return t[p0:p0 + n, b0 * BW:(b0 + nb) * BW].rearrange(
        "p (b c) -> p b c", b=nb)


def _build_nc(count_split):
    nc = bacc.Bacc("TRN2", target_bir_lowering=False, debug=False, num_devices=8)
    x = nc.dram_tensor("x", [NIMG, H, W], F32, kind="ExternalInput")
    acc_dram = nc.dram_tensor("acc", [NPAIR, 128, NBIN], F32, kind="ExternalOutput")
    acc_dram2 = nc.dram_tensor("acc2", [NPAIR, 128, NBIN], F32, kind="ExternalOutput")

    with tile.TileContext(nc) as tc:
        with tc.tile_pool(name="p", bufs=2) as pool, \
                tc.tile_pool(name="px", bufs=2) as poolx, \
                tc.tile_pool(name="pm", bufs=3) as poolm, \
                tc.tile_pool(name="pt", bufs=1) as poolt, \
                tc.tile_pool(name="ps", bufs=1, space="PSUM") as poolp:
            bias_t = pool.tile([128, NBIN], F32, tag="bias_t")
            identf = pool.tile([128, 128], F32, tag="identf")
            make_identity(nc, identf[:])
            for bi, c in enumerate(UNIS.tolist()):
                if count_split[bi] in ("a", "c"):
                    nc.vector.memset(bias_t[:, bi:bi + 1], -float(c))
            zrow = pool.tile([128, BW], I16, tag="zrow")
            nc.vector.memset(zrow[:], 0.0)
            idw = []
            for i, (_, _, w) in enumerate(NEIGHBORS):
                iw = pool.tile([128, 128], BF16, tag=f"idw{i}")
                nc.scalar.mul(iw[:], identf[:], float(w))
                idw.append(iw)
            for pr in range(NPAIR):
                xf = poolx.tile([128, CW], F32, tag="xf")
                # partitions 0..63 <- image 2*pr, 64..127 <- image 2*pr+1
                for h in range(2):
                    nc.sync.dma_start(
                        xf[64 * h:64 * (h + 1)].rearrange("p (b c) -> p b c", b=NB),
                        x.ap()[2 * pr + h].rearrange("(b p) c -> p b c", b=NB))
                # u8 value via RNE(x*255 - 0.5) -> int16 written into the
                # padded layout (pad cols stay zero from per-pair memsets)
                r16 = poolx.tile([128, FW], I16, tag="r16")
                nc.gpsimd.memset(r16[:].rearrange("p (b c) -> p b c", b=NB)[:, :, 0:OFF], 0.0)
                nc.gpsimd.memset(r16[:].rearrange("p (b c) -> p b c", b=NB)[:, :, OFF + W:BW], 0.0)
                nc.scalar.activation(out=_img3(r16, OFF),
                                     in_=xf[:].rearrange("p (b c) -> p b c", b=NB),
                                     func=mybir.ActivationFunctionType.Copy,
                                     bias=-0.5, scale=255.0)

                # row-shifted copies within each 64-partition half:
                # um3[p] = row p-3, dp3[p] = row p+3 (image-row space)
                um3 = poolx.tile([128, FW], I16, tag="um3")
                dp3 = poolx.tile([128, FW], I16, tag="dp3")
                for h in (0, 64):
                    nc.vector.memset(um3[h:h + 3, 0:BW], 0.0)          # image top
                    nc.sync.dma_start(um3[h + 3:h + 64, :], r16[h:h + 61, :])
                    nc.sync.dma_start(_blockv(um3, h, 3, 1, NB - 1),   # block wrap
                                      _blockv(r16, h + 61, 3, 0, NB - 1))
                    nc.sync.dma_start(dp3[h + 61:h + 64, (NB - 1) * BW:FW],
                                      zrow[0:3, :])
                    nc.sync.dma_start(dp3[h:h + 61, :], r16[h + 3:h + 64, :])
                    nc.sync.dma_start(_blockv(dp3, h + 61, 3, 0, NB - 1),
                                      _blockv(r16, h, 3, 1, NB - 1))

                # +1-element copies so odd-dx reads start at even (4B-aligned) cols
                sh = {}
                for nm, src in (("im1", r16), ("um31", um3), ("dp31", dp3)):
                    t1 = poolx.tile([128, FW], I16, tag=nm)
                    nc.sync.dma_start(t1[:, 0:FW - 1], src[:, 1:FW])
                    nc.gpsimd.memset(t1[:, FW - 1:FW], 0.0)
                    sh[nm] = t1
                base = {(-3, 0): um3, (-3, 1): sh["um31"], (0, 0): r16, (0, 1): sh["im1"],
                        (3, 0): dp3, (3, 1): sh["dp31"]}

                # code = sum w_i * m_i on the PE, mask-major: each mask tile is
                # consumed by 8 chunk matmuls right after DVE produces it
                cps = []
                for ch in range(NB):
                    cpst = poolp.tile([128, W], F32, tag=f"cps{ch}",
                                      name=f"cps{ch}")
                    cps.append(cpst)
                for i, (dy, dx, w) in enumerate(NEIGHBORS):
                    m = poolm.tile([128, CW], BF16, tag="m")
                    if dx % 2 == 0:
                        src_ap = _img3(base[(dy, 0)], OFF + dx)
                    else:
                        src_ap = _img3(base[(dy, 1)], OFF + dx - 1)
                    nc.vector.tensor_tensor(out=m[:].rearrange("p (b c) -> p b c", b=NB),
                                            in0=src_ap,
                                            in1=_img3(r16, OFF),
                                            op=mybir.AluOpType.is_ge)
                    for ch in range(NB):
                        nc.tensor.matmul(out=cps[ch][:], lhsT=idw[i][:],
                                         rhs=m[:, ch * W:(ch + 1) * W],
                                         start=(i == 0), stop=(i == 7))
                code = pool.tile([128, CW], BF16, tag="code")
                for ch in range(NB):
                    nc.scalar.copy(out=code[:, ch * W:(ch + 1) * W], in_=cps[ch][:])

                accb = pool.tile([128, NBIN], F32, tag="accb")
                acca = pool.tile([128, NBIN], F32, tag="acca")
                nc.gpsimd.memset(accb[:], 0.0)
                nc.gpsimd.memset(acca[:], 0.0)
                trash_v = poolt.tile([128, CW], BF16, tag="trash_v")
                trash_a = poolt.tile([128, CW], BF16, tag="trash_a")
                trash_a2 = trash_a  # 'a' bins unused in the current split
                for bi, c in enumerate(UNIS.tolist()):
                    eng = count_split[bi]
                    if eng == "v":
                        nc.vector.tensor_scalar(out=trash_v[:], in0=code[:],
                                                scalar1=float(c), scalar2=0.0,
                                                op0=mybir.AluOpType.is_equal,
                                                op1=mybir.AluOpType.add,
                                                accum_out=accb[:, bi:bi + 1])
                    elif eng == "c":
                        # cum-chain: acca[bi] = sum Sign(code - c)
                        #          = N - cum(c) - cum(c+1); solved on host
                        nc.scalar.activation(out=trash_a[:], in_=code[:],
                                             func=mybir.ActivationFunctionType.Sign,
                                             bias=bias_t[:, bi:bi + 1],
                                             accum_out=acca[:, bi:bi + 1])
                    else:  # 'a': ACT 2-pass, acca accumulates #mismatch
                        nc.scalar.activation(out=trash_a[:], in_=code[:],
                                             func=mybir.ActivationFunctionType.Sign,
                                             bias=bias_t[:, bi:bi + 1])
                        nc.scalar.activation(out=trash_a2[:], in_=trash_a[:],
                                             func=mybir.ActivationFunctionType.Square,
                                             accum_out=acca[:, bi:bi + 1])
                # anchors: accb[bi0] = cum(c0) = #(code < c0) for interior runs
                for (bi0, c0, ln, kind) in CUM_RUNS:
                    if kind == "anchor":
                        nc.vector.tensor_scalar(out=trash_v[:], in0=code[:],
                                                scalar1=float(c0), scalar2=0.0,
                                                op0=mybir.AluOpType.is_lt,
                                                op1=mybir.AluOpType.add,
                                                accum_out=accb[:, bi0:bi0 + 1])
                nc.sync.dma_start(acc_dram.ap()[pr], accb[:])
                nc.sync.dma_start(acc_dram2.ap()[pr], acca[:])
    nc.compile()
    return nc


def _get_nc(count_split):
    key = "".join(count_split)
    if key not in _CACHE:
        _CACHE[key] = _build_nc(count_split)
    return _CACHE[key]


# engine per bin (see _build_nc): cum-chain runs are UNIS[0..4] = 0..4,
# UNIS[27..30] = 126..129 (anchored by a DVE is_lt pass), UNIS[53..57] =
# 251..255; everything else on DVE.
COUNT_SPLIT = ["c"] * 5 + ["v"] * 22 + ["c"] * 4 + ["v"] * 22 + ["c"] * 5


_NB_OFF = [(0, 5, 1), (0, 8, 2), (3, 10, 4), (6, 8, 8),
           (6, 5, 16), (6, 2, 32), (3, 0, 64), (0, 2, 128)]


def _codes_at(img, ys, xs):
    """LBP codes of img (uint8-valued int32 [H,W], zero-pad semantics) at (ys, xs)."""
    p = np.pad(img, ((3, 3), (5, 5)))
    c = img[ys, xs]
    z = np.zeros_like(c)
    for dy, dx, w in _NB_OFF:
        z = z + (p[ys + dy, xs + dx] >= c).astype(np.int32) * w
    return z


def _host_fix(x, out_sums):
    """Correct out_sums [128, NBIN] (pre-mod counts of UNIS codes) for pixels
    where the device's RNE(v-0.5) differs from floor(v)."""
    v = x.astype(np.float32) * np.float32(255.0)
    r_hw = np.rint(v - np.float32(0.5)).astype(np.int32)
    u_true = np.floor(v).astype(np.int32)
    bad = np.argwhere(r_hw != u_true)
    if len(bad) == 0:
        return
    sel = np.full(256, -1, np.int32)
    sel[UNIS] = np.arange(len(UNIS))
    H_, W_ = x.shape[1:]
    for b in np.unique(bad[:, 0]):
        pix = bad[bad[:, 0] == b][:, 1:]
        pos = set()
        for (y, xx) in pix:
            pos.add((y, xx))
            for dy, dx, _ in _NB_OFF:
                ny, nx = y - (dy - 3), xx - (dx - 5)
                if 0 <= ny < H_ and 0 <= nx < W_:
                    pos.add((ny, nx))
        ys = np.array([p_[0] for p_ in pos]); xs = np.array([p_[1] for p_ in pos])
        old = _codes_at(r_hw[b], ys, xs)
        new = _codes_at(u_true[b], ys, xs)
        for code_arr, sgn in ((old, -1.0), (new, 1.0)):
            for cd in code_arr:
                if sel[cd] >= 0:
                    out_sums[b, sel[cd]] += sgn


def kernel(x: np.ndarray) -> np.ndarray:
    x = np.ascontiguousarray(x, dtype=np.float32)
    nc = _get_nc(COUNT_SPLIT)
    in_maps = [{"x": x[c * NIMG:(c + 1) * NIMG]} for c in range(8)]
    res = run_bass_kernel_spmd(nc, in_maps, list(range(8)))
    all_sums = np.zeros((8 * NIMG, NBIN), dtype=np.float64)
    for c in range(8):
        # split pair accumulators into per-image sums: partitions 0..63 are
        # image 2*pr, 64..127 image 2*pr+1 -> [NIMG, NBIN]
        sb = res.results[c]["acc"].astype(np.float64).reshape(
            NPAIR, 2, 64, NBIN).sum(axis=2).reshape(NIMG, NBIN)
        sa = res.results[c]["acc2"].astype(np.float64).reshape(
            NPAIR, 2, 64, NBIN).sum(axis=2).reshape(NIMG, NBIN)
        sums = np.zeros_like(sb)
        for bi in range(NBIN):
            if COUNT_SPLIT[bi] == "v":
                sums[:, bi] = sb[:, bi]
            elif COUNT_SPLIT[bi] == "a":     # ACT pass counted mismatches
                sums[:, bi] = NPIX - sa[:, bi]
        # cum-chain runs: A_c = NPIX - cum(c) - cum(c+1)
        runs = []
        for (bi0, c0, ln, kind) in CUM_RUNS:
            if kind == "start":
                runs.append((bi0, bi0 + ln - 1, np.zeros(sb.shape[0])))
            elif kind == "anchor":
                runs.append((bi0, bi0 + ln - 1, sb[:, bi0].copy()))
            else:
                runs.append((bi0, bi0 + ln - 1, None))
        for (bi0, bi1, cum0) in runs:
            if cum0 is not None:             # forward chain
                cum = cum0
                for bi in range(bi0, bi1 + 1):
                    cum_next = NPIX - cum - sa[:, bi]
                    sums[:, bi] = cum_next - cum
                    cum = cum_next
            else:                            # backward chain from cum(256)=NPIX
                cum_hi = np.full(sb.shape[0], NPIX)
                for bi in range(bi1, bi0 - 1, -1):
                    cum_lo = NPIX - sa[:, bi] - cum_hi
                    sums[:, bi] = cum_hi - cum_lo
                    cum_hi = cum_lo
        all_sums[c * NIMG:(c + 1) * NIMG] = sums
    _host_fix(x, all_sums)
    out = np.zeros((128, 59), dtype=np.float32)
    out[:, :NBIN] = np.mod(all_sums, 256.0)
    out[:, NBIN] = np.mod(H * W - all_sums.sum(axis=1), 256.0)
    return out
